# revision 46
# baseline (speedup 1.0000x reference)
"""Trainium2 Bass kernel for the MACE-style symmetric contraction:

    out  = einsum("xyik,kc,bci->bcxy", U3, w3, nf)
    c2   = einsum("xyk,kc->cxy", U2, w2)[None] + out
    out  = einsum("bcxi,bci->bcx", c2, nf)
    c1   = einsum("xk,kc->cx", U1, w1)[None] + out
    out  = einsum("bci,bci->bc", c1, nf)

Algebraically:

    out[b,c] =   sum_{x,y,i} W3U[x,y,i,c] nf[b,c,x] nf[b,c,y] nf[b,c,i]
               + sum_{x,y}   U2w2[c,x,y]  nf[b,c,x] nf[b,c,y]
               + sum_{x}     U1w1[c,x]    nf[b,c,x]

with W3U = einsum("xyik,kc->xyic", U3, w3).  The U2/U1 terms are folded in
by augmenting the i axis (row i'=48 holds U2w2; (i'=48, y'=48) holds U1w1)
and appending a constant-1 channel to nf (y'=48 column of nfy).

Sharding: leading irrep axis x (48) split 6-per-core across 8 NeuronCores
(splits the dominant HBM traffic, U3, 8 ways).  Each core computes a
partial [512, 96] output; the host sums the 8 partials.

Implementation notes (v2, tuned against the TRN2 cost model):
  * All matmul operands are bf16 (fp32 matmul costs 4 cycles/row, bf16 1).
  * DMA instruction count kept low (~110/core): each DMA occupies the
    shared HWDGE device ~625 ns.  All descriptor lines >= 512 B where it
    matters (smaller lines pay 2x bus time).
  * Phase A builds W3U[c, (i',x,y')] = w3^T @ u3t in PSUM, downcasts to
    bf16 via scalar/vector copies and stores to a DRAM scratch; the whole
    W3U then reloads transposed as [i' partitions, (cpair, x, y')] (two
    strided DMAs) and stays SBUF-resident.
  * Phase B per (c-pair, b-chunk): two bf16 matmuls contract i'
    (Z = nfa^T @ W3U), ACT copies Z PSUM->SBUF as bf16, DVE/Pool build
    P2 = nf_x (x) nf_y (rank-1 per row), and one fused DVE
    scalar_tensor_tensor per c computes accum_out[b] = sum(Z * P2) --
    the final out[b,c] -- at the 4x DVE rate (all-SBUF bf16 operands).
"""

import numpy as np

B = 512          # atoms
C = 96           # feats
I = 48           # irreps
K3, K2, K1 = 1270, 24, 3
NCORES = 8
XS = I // NCORES  # 6 x-values per core
Y1 = I + 1        # 49: y plus augmentation column
I1 = I + 1        # 49: i plus augmentation row
KP = 1280         # K3 padded to 10 partition tiles
KT = KP // 128    # 10
NX = XS * Y1      # 294
MU = I * NX       # 14112 u3t columns (i outermost: m = (i, x, y'))
MP = I1 * NX      # 14406 scratch columns (incl. aug row i'=48)
ECH = 1764        # build eighth width (8 * 1764 = 14112)
NE = MU // ECH    # 8
MC = 441          # psum chunk (4 * 441 = 1764, 441*4B < 2KB bank)
NMCE = ECH // MC  # 4
PAIRS = C // 2    # 48
NT = B // 128     # 4 b-chunks

_CACHE = {}

# exec time of the last device run (ns), when BASS_TRACE=1
LAST_EXEC_NS = None


def _build_nc():
    import concourse.bass as bass
    import concourse.mybir as mybir
    from concourse.tile import TileContext

    f32 = mybir.dt.float32
    bf16 = mybir.dt.bfloat16
    mult = mybir.AluOpType.mult
    add = mybir.AluOpType.add
    bypass = mybir.AluOpType.bypass

    import concourse.bacc as bacc
    nc = bacc.Bacc(None, target_bir_lowering=False)
    u3t = nc.dram_tensor("u3t", [KP, MU], bf16, kind="ExternalInput")
    w3p = nc.dram_tensor("w3p", [KP, C], bf16, kind="ExternalInput")
    nfa = nc.dram_tensor("nfa", [128, NT * PAIRS * 128], bf16,
                         kind="ExternalInput")
    nfy = nc.dram_tensor("nfy", [B, C * I1], bf16, kind="ExternalInput")
    nfx = nc.dram_tensor("nfx", [B, C * XS], bf16, kind="ExternalInput")
    u2aug = nc.dram_tensor("u2aug", [32, NX], bf16, kind="ExternalInput")
    w21 = nc.dram_tensor("w21", [32, C], bf16, kind="ExternalInput")
    outp = nc.dram_tensor("out", [B, C], f32, kind="ExternalOutput")

    with TileContext(nc) as tc:
        with (
            tc.tile_pool(name="dram", bufs=1, space="DRAM") as dpool,
            tc.tile_pool(name="const", bufs=1) as cpool,
            tc.tile_pool(name="u3", bufs=6) as u3pool,
            tc.tile_pool(name="bpsum", bufs=4, space="PSUM") as bpsum,
            tc.tile_pool(name="zpsum", bufs=2, space="PSUM") as zpsum,
            tc.tile_pool(name="stg", bufs=3) as stgpool,
            tc.tile_pool(name="lt", bufs=1) as ltpool,
            tc.tile_pool(name="zsb", bufs=4) as zsbpool,
            tc.tile_pool(name="t2", bufs=5) as t2pool,
        ):
            # resident W3U, transposed: [i' (+64*ci), (cp, x, y')].
            # Filled chunk-by-chunk during the build via a DRAM bounce
            # (write [c, m] chunk, immediately read back transposed), so
            # the relayout pipelines behind the build instead of
            # serializing after it.  c rows are host-permuted (ci, cp).
            ltall = ltpool.tile([128, PAIRS * NX], bf16)
            ltv = ltall[:, :].rearrange("p (cp xy) -> p cp xy", xy=NX)
            w3u_scr = dpool.tile([C, MU], bf16)
            w3u_aug = dpool.tile([C, NX], bf16, name="w3u_aug")
            scr_v = w3u_scr[:, :].rearrange(
                "(ci cp) (i xy) -> ci i cp xy", ci=2, xy=NX)

            # ---- w3 weights (needed by the first build matmul) ----
            w3sb = cpool.tile([128, KT * C], bf16)
            w3v = w3sb[:, :].rearrange("p (k c) -> p k c", c=C)
            nc.sync.dma_start(
                out=w3v[:, :, :],
                in_=w3p[:, :].rearrange("(k p) c -> p k c", p=128))

            # phase-B input tiles; their loads are interleaved into the
            # build loop to ride in the DMA stream's slack
            w21sb = cpool.tile([32, C], bf16)
            u2sb = cpool.tile([32, NX], bf16)
            nfasb = cpool.tile([128, NT * PAIRS * 128], bf16)
            nfav = nfasb[:, :].rearrange("p (t cp b) -> p t cp b",
                                         t=NT, b=128)
            nfyts = [cpool.tile([128, C * I1], bf16, tag=f"nfy{t}",
                                name=f"nfy{t}") for t in range(NT)]
            nfxts = [cpool.tile([128, C * XS], bf16, tag=f"nfx{t}",
                                name=f"nfx{t}") for t in range(NT)]
            yaccs = [cpool.tile([128, C * XS], f32, tag=f"ya{t}",
                                name=f"ya{t}") for t in range(NT)]
            ynts = [cpool.tile([128, C * XS], f32, tag=f"yn{t}",
                               name=f"yn{t}") for t in range(NT)]
            ostfs = [cpool.tile([128, C], f32, tag=f"os{t}",
                                name=f"os{t}") for t in range(NT)]

            # ---- phase A: W3U build, [96, 14112] = w3p.T @ u3t ----
            for e in range(NE):
                pss = [bpsum.tile([C, MC], f32, tag="bp", name=f"bp{e}_{m}")
                       for m in range(NMCE)]
                for kt in range(KT):
                    t = u3pool.tile([128, ECH], bf16, tag="u3")
                    nc.sync.dma_start(
                        out=t[:, :],
                        in_=u3t[kt * 128:(kt + 1) * 128,
                                e * ECH:(e + 1) * ECH])
                    for m in range(NMCE):
                        nc.tensor.matmul(pss[m][:, :], w3v[:, kt, :],
                                         t[:, m * MC:(m + 1) * MC],
                                         start=(kt == 0), stop=(kt == KT - 1))
                stgt = stgpool.tile([C, ECH], bf16, tag="stg")
                for m in range(NMCE):
                    if m % 2 == 0:
                        nc.scalar.copy(stgt[:, m * MC:(m + 1) * MC],
                                       pss[m][:, :])
                    else:
                        nc.vector.tensor_copy(stgt[:, m * MC:(m + 1) * MC],
                                              pss[m][:, :])
                # bounce this chunk through DRAM to transpose it into
                # ltall rows 6e..6e+6 (and +64) while the build continues
                nc.sync.dma_start(
                    out=w3u_scr[:, e * ECH:(e + 1) * ECH], in_=stgt[:, :])
                for ci in range(2):
                    nc.sync.dma_start(
                        out=ltall[64 * ci + 6 * e:64 * ci + 6 * e + 6,
                                  :].rearrange("i (cp xy) -> i cp xy",
                                               xy=NX),
                        in_=scr_v[ci, 6 * e:6 * e + 6])
                if e == 1:
                    nc.sync.dma_start(out=w21sb[:, :], in_=w21[:, :])
                    nc.sync.dma_start(out=u2sb[:, :], in_=u2aug[:, :])

            # ---- aug row: [96, 294] = w21.T @ u2aug at i'=48 ----
            aps = bpsum.tile([C, MC], f32, tag="bp", name="bpaug")
            nc.tensor.matmul(aps[:, :NX], w21sb[:27, :], u2sb[:27, :],
                             start=True, stop=True)
            astg = stgpool.tile([C, ECH], bf16, tag="stg", name="stgaug")
            nc.scalar.copy(astg[:, :NX], aps[:, :NX])
            nc.sync.dma_start(out=w3u_aug[:, :], in_=astg[:, :NX])
            aug_v = w3u_aug[:, :].rearrange("(ci cp) (i xy) -> ci i cp xy",
                                            ci=2, xy=NX)
            for ci in range(2):
                nc.sync.dma_start(
                    out=ltall[64 * ci + I:64 * ci + I1, :].rearrange(
                        "i (cp xy) -> i cp xy", xy=NX),
                    in_=aug_v[ci])

            # t=0 phase-B loads ride the build tail
            nfav4 = nfasb[:, :].rearrange("p (t f) -> p t f", t=NT)
            nc.sync.dma_start(out=nfav4[:, 0, :],
                              in_=nfa[:, 0:PAIRS * 128])
            nc.sync.dma_start(out=nfyts[0][:, :], in_=nfy[0:128, :])
            nc.sync.dma_start(out=nfxts[0][:, :], in_=nfx[0:128, :])

            # ---- phase B (t-outer so each b-chunk's nf loads stagger) ----
            for t in range(NT):
                if t > 0:
                    nc.sync.dma_start(
                        out=nfav4[:, t, :],
                        in_=nfa[:, t * PAIRS * 128:(t + 1) * PAIRS * 128])
                    nc.sync.dma_start(out=nfyts[t][:, :],
                                      in_=nfy[t * 128:(t + 1) * 128, :])
                    nc.sync.dma_start(out=nfxts[t][:, :],
                                      in_=nfx[t * 128:(t + 1) * 128, :])
                nfyv = nfyts[t][:, :].rearrange("p (c i) -> p c i", i=I1)
                sv = yaccs[t][:, :].rearrange("p (c x) -> p c x", x=XS)
                for cp in range(PAIRS):
                    zt = zpsum.tile([128, 1024], f32, tag="z")
                    for ci in range(2):
                        nc.tensor.matmul(
                            zt[:, 512 * ci:512 * ci + NX],
                            nfav[64 * ci:64 * ci + I1, t, cp, :],
                            ltv[64 * ci:64 * ci + I1, cp, :],
                            start=True, stop=True)
                    ztv = zt[:, :].rearrange(
                        "p (ci n) -> p ci n", n=512)[:, :, 0:NX]
                    zsb = zsbpool.tile([128, 2 * NX], bf16, tag="zsb")
                    zsbv = zsb[:, :].rearrange("p (ci n) -> p ci n", n=NX)
                    nc.scalar.copy(zsbv, ztv)

                    # pass 1: T'[b,(ci,x,y')] = Z * nf_y'  (bcast over x)
                    # pass 2: S[b,(c,x)] = sum_y' T'  (plain reduce)
                    # tail (per t): out[b,c] = sum_x S * nf_x
                    t2 = t2pool.tile([128, 2 * NX], bf16, tag="t2")
                    t2v = t2[:, :].rearrange("p (ci x y) -> p ci x y",
                                             ci=2, y=Y1)
                    zsb4 = zsb[:, :].rearrange("p (ci x y) -> p ci x y",
                                               ci=2, y=Y1)
                    nfy_b2 = nfyv[:, 2 * cp:2 * cp + 2, None,
                                  :].to_broadcast([128, 2, XS, Y1])
                    if (cp + t) % 4 == 0:
                        nc.vector.tensor_tensor(t2v, zsb4, nfy_b2, mult)
                    else:
                        nc.gpsimd.tensor_tensor(t2v, zsb4, nfy_b2, mult)
                    nc.vector.tensor_reduce(
                        sv[:, 2 * cp:2 * cp + 2, :], t2v,
                        axis=mybir.AxisListType.X, op=add)

                # tail: yn = S * nfx ; out[b,c] = sum_x yn
                nc.vector.tensor_tensor(ynts[t][:, :], yaccs[t][:, :],
                                        nfxts[t][:, :], mult)
                nc.vector.tensor_reduce(
                    ostfs[t][:, :],
                    ynts[t][:, :].rearrange("p (c x) -> p c x", x=XS),
                    axis=mybir.AxisListType.X, op=add)
                nc.sync.dma_start(out=outp[t * 128:(t + 1) * 128, :],
                                  in_=ostfs[t][:, :])

    nc.finalize()
    return nc


def _prep_inputs(node_feats, w3, w2, w1, U3, U2, U1):
    """Host-side sharding / re-layout. Only transposes, padding, dtype
    casts and concatenation of the raw inputs -- no contractions."""
    import ml_dtypes
    bf16 = ml_dtypes.bfloat16
    f32 = np.float32
    node_feats = np.ascontiguousarray(np.asarray(node_feats, dtype=f32))
    w3 = np.asarray(w3, dtype=f32)
    w2 = np.asarray(w2, dtype=f32)
    w1 = np.asarray(w1, dtype=f32)
    U3 = np.asarray(U3, dtype=f32)
    U2 = np.asarray(U2, dtype=f32)
    U1 = np.asarray(U1, dtype=f32)

    # shared across cores.  The c axis of the W3U build is permuted to
    # (parity, pair) order -- even c's in rows 0..47, odd in 48..95 -- so
    # the on-chip transpose DMAs see partition-contiguous ci halves.
    cperm = np.concatenate([np.arange(0, C, 2), np.arange(1, C, 2)])
    w3p = np.zeros((KP, C), dtype=f32)
    w3p[:K3] = w3
    w3p = np.ascontiguousarray(w3p[:, cperm]).astype(bf16)
    w21 = np.zeros((32, C), dtype=f32)
    w21[:K2] = w2
    w21[K2:K2 + K1] = w1
    w21 = np.ascontiguousarray(w21[:, cperm]).astype(bf16)

    # nfa: [p, t, cp, b128]; p = 64*(c%2) + i'; row i'=48 is ones
    nfT = node_feats.transpose(1, 2, 0)  # [c, i, b]
    nfa = np.zeros((128, NT, PAIRS, 128), dtype=f32)
    for par in (0, 1):
        # [i, cp, t, b128] -> place at rows 64*par + i
        blk = nfT[par::2].transpose(1, 0, 2).reshape(I, PAIRS, NT, 128)
        nfa[64 * par:64 * par + I] = blk.transpose(0, 2, 1, 3)
        nfa[64 * par + I] = 1.0
    nfa = np.ascontiguousarray(nfa.reshape(128, NT * PAIRS * 128)).astype(bf16)

    # nfy: [b, c, 49] with ones channel
    nfy = np.empty((B, C, I1), dtype=f32)
    nfy[:, :, :I] = node_feats
    nfy[:, :, I] = 1.0
    nfy = np.ascontiguousarray(nfy.reshape(B, C * I1)).astype(bf16)

    in_maps = []
    for r in range(NCORES):
        xlo = XS * r
        # u3t: [k, m], m = (i, x, y') zero-padded at y'=48 and k>=1270
        u3s = U3[xlo:xlo + XS]                      # [6, 48, 48, 1270]
        u3a = np.zeros((I, XS, Y1, KP), dtype=f32)  # [i, x, y', k]
        u3a[:, :, :I, :K3] = u3s.transpose(2, 0, 1, 3)
        u3t = np.ascontiguousarray(u3a.reshape(MU, KP).T).astype(bf16)

        # u2aug: rows 0:24 U2 slice, rows 24:27 U1 slice (at y'=48)
        u2a = np.zeros((32, XS, Y1), dtype=f32)
        u2a[:K2, :, :I] = U2[xlo:xlo + XS].transpose(2, 0, 1)
        u2a[K2:K2 + K1, :, I] = U1[xlo:xlo + XS].T
        u2a = np.ascontiguousarray(u2a.reshape(32, NX)).astype(bf16)

        # nfx: [b, c, 6] x-slice of node_feats for this core
        nfx = np.ascontiguousarray(
            node_feats[:, :, xlo:xlo + XS].reshape(B, C * XS)).astype(bf16)

        in_maps.append({
            "u3t": u3t,
            "w3p": w3p,
            "nfa": nfa,
            "nfy": nfy,
            "nfx": nfx,
            "u2aug": u2a,
            "w21": w21,
        })
    return in_maps


def kernel(node_feats, w3, w2, w1, U3, U2, U1):
    global LAST_EXEC_NS
    import os
    from concourse.bass_utils import run_bass_kernel_spmd

    if "nc" not in _CACHE:
        _CACHE["nc"] = _build_nc()
    nc = _CACHE["nc"]

    in_maps = _prep_inputs(node_feats, w3, w2, w1, U3, U2, U1)
    trace = bool(os.environ.get("BASS_TRACE"))
    res = run_bass_kernel_spmd(nc, in_maps, list(range(NCORES)), trace=trace)
    LAST_EXEC_NS = res.exec_time_ns
    _CACHE["last_results"] = res

    out = np.zeros((B, C), dtype=np.float64)
    for r in range(NCORES):
        out += res.results[r]["out"].astype(np.float64)
    return out.astype(np.float32)
